# revision 27
# baseline (speedup 1.0000x reference)
"""DFlashAttention Trainium2 kernel (8-core tensor-parallel over attention heads).

Shapes (hardcoded): D=2048, N=16 q-heads, K=8 kv-heads, H=128,
T_NOISE=2048 (query tokens), T_CTX=4096, S=6144 (kv tokens).

Sharding: core c owns q-heads {2c, 2c+1} and kv-head c (GQA groups=2).
Each core computes a partial (T, D) output (its 2 heads' slice of the
o-projection contraction); the host sums the 8 partials (TP unshard).

v2 design (all matmul operands fp16):
  - Phase A (merged QKV proj): x streamed once as fp16 in [128,16,1024]
    chunks (one DMA per chunk).  ctx chunks compute k|v (256-wide moving),
    noise chunks compute k|v|q0|q1 in one 512-wide moving matmul.
    RMS-norm + RoPE in token-partition layout, PE transpose -> kT/qT [h,s].
  - Phase C (attention): two head-streams software-pipelined per t-chunk:
    PE issue order per step p: scores(p) for both streams, then AV(p-1),
    so the PE never sits behind ACT's exp.  exp = e^(score/sqrt(H) - 6.6)
    -> fp16 (bias keeps probs in fp16 range; it cancels in normalization).
    Softmax denominators: fp16 tree-fold of prob tiles on DVE (2x mode)
    + one ones-matmul on the folded tile (kills the per-s-tile rowsum
    matmul chain of v1).  Normalization fused into the av->oT copy via a
    rank-1 broadcast matmul of 1/r.
  - Phase D (o-proj): per t-chunk, both heads accumulate into one PSUM
    bank group; DMA straight PSUM->HBM.  No vector-engine work.
"""

import sys

for _p in ("/opt/trn_rl_repo", "/root/.axon_site/_ro/trn_rl_repo"):
    if _p not in sys.path:
        sys.path.append(_p)

import math
import numpy as np

import concourse.bass as bass
import concourse.tile as tile
from concourse import bacc
from concourse import mybir
from concourse.bass_utils import run_bass_kernel_spmd
from concourse.masks import make_identity

D = 2048
N_HEADS = 16
K_HEADS = 8
H = 128
T_NOISE = 2048
T_CTX = 4096
S_ALL = T_CTX + T_NOISE          # 6144
EPS = 1e-6
ROPE_THETA = 1e6
N_CORES = 8
HEADS_PER_CORE = N_HEADS // N_CORES   # 2

P = 128                       # partition dim
HALF = H // 2                 # 64
S_TILES = S_ALL // P          # 48
T_TILES = T_NOISE // P        # 16
NOISE_TILE0 = T_CTX // P      # 32  (noise tokens are s-tiles 32..47)
D_TILES = D // P              # 16
FREE = 512
PAIR = 2 * FREE               # 1024
CHUNK = 1024                  # tokens per x DMA chunk
N_CHUNKS = S_ALL // CHUNK     # 6 (chunks 4,5 are the noise tokens)
T_CHUNKS = T_NOISE // FREE    # 4
SP_PAIRS = S_TILES // 2       # 24 score pairs per (head, t-chunk)

F32 = mybir.dt.float32
F16 = mybir.dt.float16

TWO_PI = 2.0 * math.pi
INV_SQRT_H = 1.0 / math.sqrt(H)
EXP_BIAS = -6.6               # e^(13.69-6.6)*48*1.025 < 65504 (fp16 safe)

_CACHE = {}


def _build_program(reps=1):
    nc = bacc.Bacc("TRN2", target_bir_lowering=False, debug=False,
                   num_devices=N_CORES)

    # xs[p, d, s] = x_all[s, d*128+p]  (host pre-swizzled)
    xs = nc.dram_tensor("xs", [P, D_TILES, S_ALL], F16,
                        kind="ExternalInput").ap()
    # wkvq[p, d, :] = [Wk | Wv | Wq0 | Wq1][d*128+p, :]
    wkvq = nc.dram_tensor("wkvq", [P, D_TILES, 4 * H], F16,
                          kind="ExternalInput").ap()
    # wob[p, j, :] = Wo[head j][p, :]
    wob = nc.dram_tensor("wob", [P, HEADS_PER_CORE, D], F16,
                         kind="ExternalInput").ap()
    # host-precomputed RoPE tables: sint[p, si*64+j] = sin(pos[si*128+p]*invf[j])
    sint = nc.dram_tensor("sint", [P, S_TILES * HALF], F16,
                          kind="ExternalInput").ap()
    cost = nc.dram_tensor("cost", [P, S_TILES * HALF], F16,
                          kind="ExternalInput").ap()
    qscaleb = nc.dram_tensor("qscaleb", [P, H], F32,
                             kind="ExternalInput").ap()
    kscaleb = nc.dram_tensor("kscaleb", [P, H], F32,
                             kind="ExternalInput").ap()
    out = nc.dram_tensor("out", [T_NOISE, D], F16, kind="ExternalOutput").ap()

    with tile.TileContext(nc) as tc:
        for rep in range(reps):
            _emit(nc, tc, xs, wkvq, wob, sint, cost, qscaleb, kscaleb,
                  out, pfx=f"r{rep}_")
    nc.compile()
    return nc, "out"


def _emit(nc, tc, xs, wkvq, wob, sint, cost, qscaleb, kscaleb, out, pfx=""):
    import contextlib
    ctx = contextlib.ExitStack()
    with ctx:
        xp = ctx.enter_context(tc.tile_pool(name=pfx + "pa_x", bufs=2))
        const = ctx.enter_context(tc.tile_pool(name=pfx + "const", bufs=1))
        persist = ctx.enter_context(tc.tile_pool(name=pfx + "persist", bufs=1))

        # ---- chunk-0 x staging issued FIRST so phase A starts asap ----
        xst0 = [xp.tile([P, D_TILES // 2, CHUNK], F16, tag=f"xst{hf}",
                        name=f"xst{hf}") for hf in range(2)]
        for hf in range(2):
            nc.sync.dma_start(xst0[hf][:], xs[:, hf * 8:(hf + 1) * 8, 0:CHUNK])
        # weights in quarters so the first d-tiles' matmuls start early
        wkvq_sb = const.tile([P, D_TILES * 4 * H], F16, tag="wkvq")
        QW = D_TILES * 4 * H // 4
        for g in range(4):
            nc.sync.dma_start(wkvq_sb[:, g * QW:(g + 1) * QW],
                              wkvq[:, g * 4:(g + 1) * 4, :])

        # ---- constants ----
        ident = const.tile([P, P], F16, tag="ident")
        make_identity(nc, ident[:])
        ones16 = const.tile([P, 1], F16, tag="ones16")
        nc.vector.memset(ones16[:], 1.0)
        ones_row = const.tile([1, P], F32, tag="ones_row")
        nc.vector.memset(ones_row[:], 1.0)
        qsc_sb = const.tile([P, H], F32, tag="qsc")
        nc.sync.dma_start(qsc_sb[:], qscaleb[:])
        ksc_sb = const.tile([P, H], F32, tag="ksc")
        nc.sync.dma_start(ksc_sb[:], kscaleb[:])
        eps_col = const.tile([P, 1], F32, tag="eps")
        nc.vector.memset(eps_col[:], EPS)
        ebias_col = const.tile([P, 1], F32, tag="ebias")
        nc.vector.memset(ebias_col[:], EXP_BIAS)
        wo_sb = const.tile([P, HEADS_PER_CORE * D], F16, tag="wo")
        nc.sync.dma_start(wo_sb[:], wob[:])

        # ---- persistent activations ----
        sin_all = persist.tile([P, S_TILES * HALF], F16, tag="sin")
        cos_all = persist.tile([P, S_TILES * HALF], F16, tag="cos")
        nc.sync.dma_start(sin_all[:], sint[:])
        nc.sync.dma_start(cos_all[:], cost[:])
        kT_sb = persist.tile([P, S_ALL], F16, tag="kT")
        v_sb = persist.tile([P, S_ALL], F16, tag="v")     # [s-tile, h] blocks
        qT_sb = persist.tile([P, HEADS_PER_CORE * T_NOISE], F16, tag="qT")
        oT_sb = persist.tile([P, HEADS_PER_CORE * T_NOISE], F16, tag="oT")

        def norm_rope_transpose(src_psum, scale_sb, si, dst_sb, work, psum_t):
            """src_psum [P(tok),H] f32 -> rms-norm*scale -> rope -> transpose
            -> dst_sb [P(h), 128 tok] fp16. si = token-tile for positions."""
            sq = work.tile([P, H], F32, tag="sq")
            ssq = work.tile([P, 1], F32, tag="ssq")
            nc.scalar.activation(sq[:], src_psum,
                                 mybir.ActivationFunctionType.Square,
                                 accum_out=ssq[:])
            rms = work.tile([P, 1], F32, tag="rms")
            nc.scalar.activation(rms[:], ssq[:],
                                 mybir.ActivationFunctionType.Sqrt,
                                 bias=eps_col[:], scale=1.0 / H)
            rinv = work.tile([P, 1], F32, tag="rinv")
            nc.vector.reciprocal(rinv[:], rms[:])
            xn = work.tile([P, H], F16, tag="xn")
            nc.vector.scalar_tensor_tensor(
                xn[:], src_psum, rinv[:], scale_sb[:],
                mybir.AluOpType.mult, mybir.AluOpType.mult)
            co = cos_all[:, si * HALF:(si + 1) * HALF]
            sn = sin_all[:, si * HALF:(si + 1) * HALF]
            x1 = xn[:, 0:HALF]
            x2 = xn[:, HALF:H]
            t1 = work.tile([P, HALF], F16, tag="t1")
            t2 = work.tile([P, HALF], F16, tag="t2")
            xr = work.tile([P, H], F16, tag="xr")
            nc.vector.tensor_mul(t1[:], x1, co)
            nc.vector.tensor_mul(t2[:], x2, sn)
            nc.vector.tensor_sub(xr[:, 0:HALF], t1[:], t2[:])
            nc.vector.tensor_mul(t1[:], x2, co)
            nc.vector.tensor_mul(t2[:], x1, sn)
            nc.vector.tensor_add(xr[:, HALF:H], t1[:], t2[:])
            pt = psum_t.tile([P, P], F16, tag="pt")
            nc.tensor.transpose(pt[:], xr[:], ident[:])
            nc.scalar.copy(dst_sb, pt[:])

        # ---- Phase A: merged kvq projection -> kT, v, qT ----
        with tc.tile_pool(name=pfx + "pa_ps", bufs=2, space="PSUM") as pska, \
             tc.tile_pool(name=pfx + "pa_pt", bufs=3, space="PSUM") as pst, \
             tc.tile_pool(name=pfx + "pa_w", bufs=4) as work:
            def consume_pair(ps, si0, noi):
                for u in range(2):
                    si = si0 + u
                    nc.vector.tensor_copy(
                        v_sb[:, si * P:(si + 1) * P], ps[u][:, H:2 * H])
                    norm_rope_transpose(
                        ps[u][:, 0:H], ksc_sb, si,
                        kT_sb[:, si * P:(si + 1) * P], work, pst)
                    if noi:
                        ti = si - NOISE_TILE0
                        for hh in range(HEADS_PER_CORE):
                            norm_rope_transpose(
                                ps[u][:, (2 + hh) * H:(3 + hh) * H],
                                qsc_sb, si,
                                qT_sb[:, hh * T_NOISE + ti * P:
                                      hh * T_NOISE + (ti + 1) * P],
                                work, pst)

            pending = None
            for c in range(N_CHUNKS):
                noise = c >= 4
                W = 4 * H if noise else 2 * H
                # x chunk staged in two halves so the first matmuls can
                # start before the whole chunk has landed
                if c == 0:
                    xst = xst0
                else:
                    xst = [xp.tile([P, D_TILES // 2, CHUNK], F16,
                                   tag=f"xst{hf}", name=f"xst{hf}")
                           for hf in range(2)]
                    for hf in range(2):
                        nc.sync.dma_start(
                            xst[hf][:],
                            xs[:, hf * 8:(hf + 1) * 8,
                               c * CHUNK:(c + 1) * CHUNK])
                # 8 token-tiles per chunk, processed in pairs (2-deep PSUM).
                # Consumption lags the matmul bursts by one pair so the
                # norm/rope/transpose chain never blocks the mm stream.
                for pairi in range(4):
                    ps = [pska.tile([P, 4 * H], F32, tag=f"ps{u}",
                                    name=f"ps{u}") for u in range(2)]
                    for d in range(D_TILES):
                        for u in range(2):
                            tok = pairi * 2 + u
                            nc.tensor.matmul(
                                ps[u][:, 0:W],
                                xst[d // 8][:, d % 8, tok * P:(tok + 1) * P],
                                wkvq_sb[:, d * 4 * H:d * 4 * H + W],
                                start=(d == 0), stop=(d == D_TILES - 1))
                    if pending is not None:
                        consume_pair(*pending)
                    pending = (ps, c * 8 + pairi * 2, noise)
            consume_pair(*pending)

        # ---- Phase C + D: attention (2 head-streams) + o-projection ----
        # PSUM: one rotating 3-deep [P,1024] pool (6 banks) shared by the
        # score tiles of both streams, D's po pairs, and the epilogue rbc;
        # + av0,av1 accumulators (1 bank each).
        with tc.tile_pool(name=pfx + "pc_big", bufs=3, space="PSUM") as pbig, \
             tc.tile_pool(name=pfx + "pc_av", bufs=1, space="PSUM") as pav, \
             tc.tile_pool(name=pfx + "pc_ex", bufs=6) as pexp, \
             tc.tile_pool(name=pfx + "pc_rf", bufs=2) as prf, \
             tc.tile_pool(name=pfx + "pc_rv", bufs=2) as prv, \
             tc.tile_pool(name=pfx + "pc_ob", bufs=4) as posb:

            def emit_d_piece(dtch, ti, dh):
                """o-projection for t-tile ti, D-half dh of t-chunk dtch."""
                t0 = dtch * FREE + ti * P
                po = pbig.tile([P, PAIR], F32, tag="big", name="po")
                for st in range(2):
                    osl = oT_sb[:, st * T_NOISE + t0:st * T_NOISE + t0 + P]
                    for u in range(2):
                        nc.tensor.matmul(
                            po[:, u * FREE:(u + 1) * FREE], osl,
                            wo_sb[:, st * D + dh * PAIR + u * FREE:
                                  st * D + dh * PAIR + (u + 1) * FREE],
                            start=(st == 0), stop=(st == 1))
                for u in range(2):
                    ob = posb.tile([P, FREE], F16, tag="ob", name="ob")
                    nc.vector.tensor_copy(ob[:], po[:, u * FREE:(u + 1) * FREE])
                    nc.sync.dma_start(
                        out[t0:t0 + P,
                            dh * PAIR + u * FREE:dh * PAIR + (u + 1) * FREE],
                        ob[:])

            def emit_scores_exp(p, qsl, cur_ex):
                # scores for pair p; stationary kT tile shared between the
                # two streams (u-outer).  Fresh score tiles from the
                # rotating pool: the PE never waits for exp's read of the
                # previous pair.
                scp = [pbig.tile([P, PAIR], F32, tag="big",
                                 name=f"sc{st}") for st in range(2)]
                for u in range(2):
                    si = 2 * p + u
                    for st in range(2):
                        nc.tensor.matmul(
                            scp[st][:, u * FREE:(u + 1) * FREE],
                            kT_sb[:, si * P:(si + 1) * P], qsl[st],
                            start=True, stop=True)
                for st in range(2):
                    e = pexp.tile([P, PAIR], F16, tag=f"ex{st}",
                                  name=f"ex{st}")
                    nc.scalar.activation(
                        e[:], scp[st][:],
                        mybir.ActivationFunctionType.Exp,
                        bias=ebias_col[:], scale=INV_SQRT_H)
                    cur_ex[st] = e

            def emit_epilogue(etch, av, rfa, rfb):
                # per stream: denominators + normalized oT
                for st in range(2):
                    rbct = pbig.tile([P, PAIR], F32, tag="big", name="rbct")
                    rbc = rbct[:, 0:FREE]
                    nc.tensor.matmul(rbc[0:1, :], ones16[:],
                                     rfa[st][:, 0:FREE],
                                     start=True, stop=False)
                    nc.tensor.matmul(rbc[0:1, :], ones16[:],
                                     rfa[st][:, FREE:PAIR],
                                     start=False, stop=False)
                    nc.tensor.matmul(rbc[0:1, :], ones16[:],
                                     rfb[st][:, 0:FREE],
                                     start=False, stop=False)
                    nc.tensor.matmul(rbc[0:1, :], ones16[:],
                                     rfb[st][:, FREE:PAIR],
                                     start=False, stop=True)
                    rinv_r = prv.tile([1, FREE], F32, tag="rinv_r",
                                      name="rinv_r")
                    nc.vector.reciprocal(rinv_r[:], rbc[0:1, :])
                    nc.tensor.matmul(rbc[:, :], ones_row[:], rinv_r[:],
                                     start=True, stop=True)
                    rbs = prv.tile([P, FREE], F32, tag="rbs", name="rbs")
                    nc.scalar.copy(rbs[:], rbc[:, :])
                    nc.vector.tensor_mul(
                        oT_sb[:, st * T_NOISE + etch * FREE:
                              st * T_NOISE + (etch + 1) * FREE],
                        av[st][:], rbs[:])
                d_queue.extend((etch, ti, dh)
                               for ti in range(4) for dh in range(2))

            d_queue = []
            pend_epi = None
            for tch in range(T_CHUNKS):
                av = [pav.tile([P, FREE], F32, tag=f"av{st}", name=f"av{st}")
                      for st in range(2)]
                # two denominator accumulators per stream: rfa on DVE,
                # rfb on the (otherwise idle) Pool engine
                rfa = [prf.tile([P, PAIR], F16, tag=f"rfa{st}",
                                name=f"rfa{st}") for st in range(2)]
                rfb = [prf.tile([P, PAIR], F16, tag=f"rfb{st}",
                                name=f"rfb{st}") for st in range(2)]
                qsl = [qT_sb[:, st * T_NOISE + tch * FREE:
                             st * T_NOISE + (tch + 1) * FREE]
                       for st in range(2)]
                prev_ex = [None, None]
                cur_ex = [None, None]
                na = [0, 0]
                nb = [0, 0]
                # hoisted first pair: ACT chews exp(0) while the previous
                # chunk's epilogue chain runs
                emit_scores_exp(0, qsl, cur_ex)
                prev_ex = list(cur_ex)
                if pend_epi is not None:
                    emit_epilogue(*pend_epi)
                for p in range(1, SP_PAIRS + 1):
                    if p < SP_PAIRS:
                        emit_scores_exp(p, qsl, cur_ex)
                    q = p - 1
                    for u in range(2):
                        si = 2 * q + u
                        for st in range(2):
                            nc.tensor.matmul(
                                av[st][:],
                                v_sb[:, si * P:(si + 1) * P],
                                prev_ex[st][:, u * FREE:(u + 1) * FREE],
                                start=(q == 0 and u == 0),
                                stop=(q == SP_PAIRS - 1 and u == 1))
                    for st in range(2):
                        e = prev_ex[st]
                        # Pool engine takes a third of the fold chain, early
                        # folds only (its op latency must not delay the
                        # epilogue's denominator matmuls)
                        if q % 3 == 1 and q <= 19:
                            if nb[st] == 0:
                                nc.gpsimd.tensor_copy(rfb[st][:], e[:])
                            else:
                                nc.gpsimd.tensor_add(rfb[st][:],
                                                     rfb[st][:], e[:])
                            nb[st] += 1
                        else:               # DVE accumulator
                            if na[st] == 0:
                                nc.vector.tensor_copy(rfa[st][:], e[:])
                            else:
                                nc.vector.tensor_add(rfa[st][:],
                                                     rfa[st][:], e[:])
                            na[st] += 1
                    # interleave previous chunk's o-projection
                    if d_queue and p >= 2 and p % 3 == 2:
                        emit_d_piece(*d_queue.pop(0))
                    prev_ex = list(cur_ex)
                pend_epi = (tch, av, rfa, rfb)
            emit_epilogue(*pend_epi)
            for piece in d_queue:
                emit_d_piece(*piece)


def _get_program(reps=1):
    key = f"prog{reps}"
    if key not in _CACHE:
        _CACHE[key] = _build_program(reps)
    return _CACHE[key]


def prepare_in_maps(x_noise, target_hidden, Wq, Wk, Wv, Wo, q_scale, k_scale,
                    noise_positions, ctx_positions):
    x_noise = np.asarray(x_noise, dtype=np.float32)
    target_hidden = np.asarray(target_hidden, dtype=np.float32)
    Wq = np.asarray(Wq, dtype=np.float32)
    Wk = np.asarray(Wk, dtype=np.float32)
    Wv = np.asarray(Wv, dtype=np.float32)
    Wo = np.asarray(Wo, dtype=np.float32)
    q_scale = np.asarray(q_scale, dtype=np.float32)
    k_scale = np.asarray(k_scale, dtype=np.float32)

    x_all = np.concatenate([target_hidden, x_noise], axis=0)       # (S, D)
    # xs[p, d, s] = x_all[s, d*128+p]
    xs = np.ascontiguousarray(
        x_all.T.reshape(D_TILES, P, S_ALL).transpose(1, 0, 2)
    ).astype(np.float16)
    pos_all = np.concatenate(
        [np.asarray(ctx_positions), np.asarray(noise_positions)]
    ).astype(np.float64)
    inv_freq = ROPE_THETA ** (-np.arange(HALF, dtype=np.float64) * 2.0 / H)
    ang = pos_all[:, None] * inv_freq[None, :]                     # (S, 64)
    # sint[p, si*64+j] = sin(pos[si*128+p] * invf[j])
    sint = np.ascontiguousarray(
        np.sin(ang).reshape(S_TILES, P, HALF).transpose(1, 0, 2)
        .reshape(P, S_TILES * HALF)).astype(np.float16)
    cost = np.ascontiguousarray(
        np.cos(ang).reshape(S_TILES, P, HALF).transpose(1, 0, 2)
        .reshape(P, S_TILES * HALF)).astype(np.float16)
    qscaleb = np.ascontiguousarray(np.broadcast_to(q_scale, (P, H)))
    kscaleb = np.ascontiguousarray(np.broadcast_to(k_scale, (P, H)))

    in_maps = []
    for c in range(N_CORES):
        wkvq = np.concatenate(
            [Wk[:, c, :], Wv[:, c, :],
             Wq[:, 2 * c, :], Wq[:, 2 * c + 1, :]], axis=1)        # (D, 512)
        wkvq = np.ascontiguousarray(
            wkvq.reshape(D_TILES, P, 4 * H).transpose(1, 0, 2)
        ).astype(np.float16)                                        # (P,16,512)
        wob = np.ascontiguousarray(
            Wo[2 * c:2 * c + 2].transpose(1, 0, 2)
        ).astype(np.float16)                                        # (P,2,D)
        in_maps.append({
            "xs": xs, "wkvq": wkvq, "wob": wob,
            "sint": sint, "cost": cost,
            "qscaleb": qscaleb, "kscaleb": kscaleb,
        })
    return in_maps


def kernel(**inputs):
    in_maps = prepare_in_maps(**inputs)
    nc, out_name = _get_program()
    res = run_bass_kernel_spmd(nc, in_maps, core_ids=list(range(N_CORES)))
    acc = np.zeros((T_NOISE, D), dtype=np.float32)
    for r in res.results:
        acc += r[out_name].astype(np.float32)
    return acc


def run_traced(inputs, **kw):
    """Run once with NTFF tracing; returns BassKernelResults (exec_time_ns)."""
    in_maps = prepare_in_maps(**inputs)
    nc, out_name = _get_program()
    return run_bass_kernel_spmd(nc, in_maps, core_ids=list(range(N_CORES)),
                                trace=True, **kw)


# revision 31
# speedup vs baseline: 1.0122x; 1.0122x over previous
"""DFlashAttention Trainium2 kernel (8-core tensor-parallel over attention heads).

Shapes (hardcoded): D=2048, N=16 q-heads, K=8 kv-heads, H=128,
T_NOISE=2048 (query tokens), T_CTX=4096, S=6144 (kv tokens).

Sharding: core c owns q-heads {2c, 2c+1} and kv-head c (GQA groups=2).
Each core computes a partial (T, D) output (its 2 heads' slice of the
o-projection contraction); the host sums the 8 partials (TP unshard).

v2 design (all matmul operands fp16):
  - Phase A (merged QKV proj): x streamed once as fp16 in [128,16,1024]
    chunks (one DMA per chunk).  ctx chunks compute k|v (256-wide moving),
    noise chunks compute k|v|q0|q1 in one 512-wide moving matmul.
    RMS-norm + RoPE in token-partition layout, PE transpose -> kT/qT [h,s].
  - Phase C (attention): two head-streams software-pipelined per t-chunk:
    PE issue order per step p: scores(p) for both streams, then AV(p-1),
    so the PE never sits behind ACT's exp.  exp = e^(score/sqrt(H) - 6.6)
    -> fp16 (bias keeps probs in fp16 range; it cancels in normalization).
    Softmax denominators: fp16 tree-fold of prob tiles on DVE (2x mode)
    + one ones-matmul on the folded tile (kills the per-s-tile rowsum
    matmul chain of v1).  Normalization fused into the av->oT copy via a
    rank-1 broadcast matmul of 1/r.
  - Phase D (o-proj): per t-chunk, both heads accumulate into one PSUM
    bank group; DMA straight PSUM->HBM.  No vector-engine work.
"""

import sys

for _p in ("/opt/trn_rl_repo", "/root/.axon_site/_ro/trn_rl_repo"):
    if _p not in sys.path:
        sys.path.append(_p)

import math
import numpy as np

import concourse.bass as bass
import concourse.tile as tile
from concourse import bacc
from concourse import mybir
from concourse.bass_utils import run_bass_kernel_spmd
from concourse.masks import make_identity

D = 2048
N_HEADS = 16
K_HEADS = 8
H = 128
T_NOISE = 2048
T_CTX = 4096
S_ALL = T_CTX + T_NOISE          # 6144
EPS = 1e-6
ROPE_THETA = 1e6
N_CORES = 8
HEADS_PER_CORE = N_HEADS // N_CORES   # 2

P = 128                       # partition dim
HALF = H // 2                 # 64
S_TILES = S_ALL // P          # 48
T_TILES = T_NOISE // P        # 16
NOISE_TILE0 = T_CTX // P      # 32  (noise tokens are s-tiles 32..47)
D_TILES = D // P              # 16
FREE = 512
PAIR = 2 * FREE               # 1024
CHUNK = 1024                  # tokens per x DMA chunk
N_CHUNKS = S_ALL // CHUNK     # 6 (chunks 4,5 are the noise tokens)
T_CHUNKS = T_NOISE // FREE    # 4
SP_PAIRS = S_TILES // 2       # 24 score pairs per (head, t-chunk)

F32 = mybir.dt.float32
F16 = mybir.dt.float16

TWO_PI = 2.0 * math.pi
INV_SQRT_H = 1.0 / math.sqrt(H)
EXP_BIAS = -6.6               # e^(13.69-6.6)*48*1.025 < 65504 (fp16 safe)

_CACHE = {}


def _build_program(reps=1):
    nc = bacc.Bacc("TRN2", target_bir_lowering=False, debug=False,
                   num_devices=N_CORES)

    # xs[p, d, s] = x_all[s, d*128+p]  (host pre-swizzled)
    xs = nc.dram_tensor("xs", [P, D_TILES, S_ALL], F16,
                        kind="ExternalInput").ap()
    # wkvq[p, d, :] = [Wk | Wv | Wq0 | Wq1][d*128+p, :]
    wkvq = nc.dram_tensor("wkvq", [P, D_TILES, 4 * H], F16,
                          kind="ExternalInput").ap()
    # wob[p, j, :] = Wo[head j][p, :]
    wob = nc.dram_tensor("wob", [P, HEADS_PER_CORE, D], F16,
                         kind="ExternalInput").ap()
    # host-precomputed RoPE tables: sint[p, si*64+j] = sin(pos[si*128+p]*invf[j])
    sint = nc.dram_tensor("sint", [P, S_TILES * HALF], F16,
                          kind="ExternalInput").ap()
    cost = nc.dram_tensor("cost", [P, S_TILES * HALF], F16,
                          kind="ExternalInput").ap()
    qscaleb = nc.dram_tensor("qscaleb", [P, H], F32,
                             kind="ExternalInput").ap()
    kscaleb = nc.dram_tensor("kscaleb", [P, H], F32,
                             kind="ExternalInput").ap()
    out = nc.dram_tensor("out", [T_NOISE, D], F16, kind="ExternalOutput").ap()

    with tile.TileContext(nc) as tc:
        for rep in range(reps):
            _emit(nc, tc, xs, wkvq, wob, sint, cost, qscaleb, kscaleb,
                  out, pfx=f"r{rep}_")
    nc.compile()
    return nc, "out"


def _emit(nc, tc, xs, wkvq, wob, sint, cost, qscaleb, kscaleb, out, pfx=""):
    import contextlib
    ctx = contextlib.ExitStack()
    with ctx:
        xp = ctx.enter_context(tc.tile_pool(name=pfx + "pa_x", bufs=2))
        const = ctx.enter_context(tc.tile_pool(name=pfx + "const", bufs=1))
        persist = ctx.enter_context(tc.tile_pool(name=pfx + "persist", bufs=1))

        # ---- chunk-0 x staging issued FIRST so phase A starts asap ----
        xst0 = [xp.tile([P, D_TILES // 2, CHUNK], F16, tag=f"xst{hf}",
                        name=f"xst{hf}") for hf in range(2)]
        for hf in range(2):
            nc.sync.dma_start(xst0[hf][:], xs[:, hf * 8:(hf + 1) * 8, 0:CHUNK])
        # weights in quarters so the first d-tiles' matmuls start early
        wkvq_sb = const.tile([P, D_TILES * 4 * H], F16, tag="wkvq")
        QW = D_TILES * 4 * H // 4
        for g in range(4):
            nc.sync.dma_start(wkvq_sb[:, g * QW:(g + 1) * QW],
                              wkvq[:, g * 4:(g + 1) * 4, :])

        # ---- constants ----
        ident = const.tile([P, P], F16, tag="ident")
        make_identity(nc, ident[:])
        ones16 = const.tile([P, 1], F16, tag="ones16")
        nc.vector.memset(ones16[:], 1.0)
        ones_row = const.tile([1, P], F32, tag="ones_row")
        nc.vector.memset(ones_row[:], 1.0)
        qsc_sb = const.tile([P, H], F32, tag="qsc")
        nc.sync.dma_start(qsc_sb[:], qscaleb[:])
        ksc_sb = const.tile([P, H], F32, tag="ksc")
        nc.sync.dma_start(ksc_sb[:], kscaleb[:])
        eps_col = const.tile([P, 1], F32, tag="eps")
        nc.vector.memset(eps_col[:], EPS)
        ebias_col = const.tile([P, 1], F32, tag="ebias")
        nc.vector.memset(ebias_col[:], EXP_BIAS)
        wo_sb = const.tile([P, HEADS_PER_CORE * D], F16, tag="wo")
        nc.sync.dma_start(wo_sb[:], wob[:])

        # ---- persistent activations ----
        sin_all = persist.tile([P, S_TILES * HALF], F16, tag="sin")
        cos_all = persist.tile([P, S_TILES * HALF], F16, tag="cos")
        nc.sync.dma_start(sin_all[:], sint[:])
        nc.sync.dma_start(cos_all[:], cost[:])
        kT_sb = persist.tile([P, S_ALL], F16, tag="kT")
        v_sb = persist.tile([P, S_ALL], F16, tag="v")     # [s-tile, h] blocks
        qT_sb = persist.tile([P, HEADS_PER_CORE * T_NOISE], F16, tag="qT")
        oT_sb = persist.tile([P, HEADS_PER_CORE * T_NOISE], F16, tag="oT")

        def norm_rope_transpose(src_psum, scale_sb, si, dst_sb, work, psum_t):
            """src_psum [P(tok),H] f32 -> rms-norm*scale -> rope -> transpose
            -> dst_sb [P(h), 128 tok] fp16. si = token-tile for positions."""
            sq = work.tile([P, H], F32, tag="sq")
            ssq = work.tile([P, 1], F32, tag="ssq")
            nc.scalar.activation(sq[:], src_psum,
                                 mybir.ActivationFunctionType.Square,
                                 accum_out=ssq[:])
            rms = work.tile([P, 1], F32, tag="rms")
            nc.scalar.activation(rms[:], ssq[:],
                                 mybir.ActivationFunctionType.Sqrt,
                                 bias=eps_col[:], scale=1.0 / H)
            rinv = work.tile([P, 1], F32, tag="rinv")
            nc.vector.reciprocal(rinv[:], rms[:])
            xn = work.tile([P, H], F16, tag="xn")
            nc.vector.scalar_tensor_tensor(
                xn[:], src_psum, rinv[:], scale_sb[:],
                mybir.AluOpType.mult, mybir.AluOpType.mult)
            co = cos_all[:, si * HALF:(si + 1) * HALF]
            sn = sin_all[:, si * HALF:(si + 1) * HALF]
            x1 = xn[:, 0:HALF]
            x2 = xn[:, HALF:H]
            t1 = work.tile([P, HALF], F16, tag="t1")
            t2 = work.tile([P, HALF], F16, tag="t2")
            xr = work.tile([P, H], F16, tag="xr")
            nc.vector.tensor_mul(t1[:], x1, co)
            nc.vector.tensor_mul(t2[:], x2, sn)
            nc.vector.tensor_sub(xr[:, 0:HALF], t1[:], t2[:])
            nc.vector.tensor_mul(t1[:], x2, co)
            nc.vector.tensor_mul(t2[:], x1, sn)
            nc.vector.tensor_add(xr[:, HALF:H], t1[:], t2[:])
            pt = psum_t.tile([P, P], F16, tag="pt")
            nc.tensor.transpose(pt[:], xr[:], ident[:])
            nc.scalar.copy(dst_sb, pt[:])

        # ---- Phase A: merged kvq projection -> kT, v, qT ----
        with tc.tile_pool(name=pfx + "pa_ps", bufs=2, space="PSUM") as pska, \
             tc.tile_pool(name=pfx + "pa_pt", bufs=3, space="PSUM") as pst, \
             tc.tile_pool(name=pfx + "pa_w", bufs=4) as work:
            def consume_pair(ps, si0, noi):
                for u in range(2):
                    si = si0 + u
                    nc.vector.tensor_copy(
                        v_sb[:, si * P:(si + 1) * P], ps[u][:, H:2 * H])
                    norm_rope_transpose(
                        ps[u][:, 0:H], ksc_sb, si,
                        kT_sb[:, si * P:(si + 1) * P], work, pst)
                    if noi:
                        ti = si - NOISE_TILE0
                        for hh in range(HEADS_PER_CORE):
                            norm_rope_transpose(
                                ps[u][:, (2 + hh) * H:(3 + hh) * H],
                                qsc_sb, si,
                                qT_sb[:, hh * T_NOISE + ti * P:
                                      hh * T_NOISE + (ti + 1) * P],
                                work, pst)

            for c in range(N_CHUNKS):
                noise = c >= 4
                W = 4 * H if noise else 2 * H
                # x chunk staged in two halves so the first matmuls can
                # start before the whole chunk has landed
                if c == 0:
                    xst = xst0
                else:
                    xst = [xp.tile([P, D_TILES // 2, CHUNK], F16,
                                   tag=f"xst{hf}", name=f"xst{hf}")
                           for hf in range(2)]
                    for hf in range(2):
                        nc.sync.dma_start(
                            xst[hf][:],
                            xs[:, hf * 8:(hf + 1) * 8,
                               c * CHUNK:(c + 1) * CHUNK])
                # 8 token-tiles per chunk, processed in pairs (2-deep PSUM).
                # Consumption lags the matmul bursts by one pair so the
                # norm/rope/transpose chain never blocks the mm stream.
                for pairi in range(4):
                    ps = [pska.tile([P, 4 * H], F32, tag=f"ps{u}",
                                    name=f"ps{u}") for u in range(2)]
                    for d in range(D_TILES):
                        for u in range(2):
                            tok = pairi * 2 + u
                            nc.tensor.matmul(
                                ps[u][:, 0:W],
                                xst[d // 8][:, d % 8, tok * P:(tok + 1) * P],
                                wkvq_sb[:, d * 4 * H:d * 4 * H + W],
                                start=(d == 0), stop=(d == D_TILES - 1))
                    consume_pair(ps, c * 8 + pairi * 2, noise)

        # ---- Phase C + D: attention (2 head-streams) + o-projection ----
        # PSUM: one rotating 3-deep [P,1024] pool (6 banks) shared by the
        # score tiles of both streams, D's po pairs, and the epilogue rbc;
        # + av0,av1 accumulators (1 bank each).
        with tc.tile_pool(name=pfx + "pc_big", bufs=3, space="PSUM") as pbig, \
             tc.tile_pool(name=pfx + "pc_av", bufs=1, space="PSUM") as pav, \
             tc.tile_pool(name=pfx + "pc_ex", bufs=6) as pexp, \
             tc.tile_pool(name=pfx + "pc_rf", bufs=2) as prf, \
             tc.tile_pool(name=pfx + "pc_rv", bufs=2) as prv, \
             tc.tile_pool(name=pfx + "pc_ob", bufs=4) as posb:

            def emit_d_piece(dtch, ti, dh):
                """o-projection for t-tile ti, D-half dh of t-chunk dtch."""
                t0 = dtch * FREE + ti * P
                po = pbig.tile([P, PAIR], F32, tag="big", name="po")
                for st in range(2):
                    osl = oT_sb[:, st * T_NOISE + t0:st * T_NOISE + t0 + P]
                    for u in range(2):
                        nc.tensor.matmul(
                            po[:, u * FREE:(u + 1) * FREE], osl,
                            wo_sb[:, st * D + dh * PAIR + u * FREE:
                                  st * D + dh * PAIR + (u + 1) * FREE],
                            start=(st == 0), stop=(st == 1))
                for u in range(2):
                    ob = posb.tile([P, FREE], F16, tag="ob", name="ob")
                    nc.vector.tensor_copy(ob[:], po[:, u * FREE:(u + 1) * FREE])
                    nc.sync.dma_start(
                        out[t0:t0 + P,
                            dh * PAIR + u * FREE:dh * PAIR + (u + 1) * FREE],
                        ob[:])

            def emit_scores_exp(p, qsl, cur_ex):
                # scores for pair p; stationary kT tile shared between the
                # two streams (u-outer).  Fresh score tiles from the
                # rotating pool: the PE never waits for exp's read of the
                # previous pair.
                scp = [pbig.tile([P, PAIR], F32, tag="big",
                                 name=f"sc{st}") for st in range(2)]
                for u in range(2):
                    si = 2 * p + u
                    for st in range(2):
                        nc.tensor.matmul(
                            scp[st][:, u * FREE:(u + 1) * FREE],
                            kT_sb[:, si * P:(si + 1) * P], qsl[st],
                            start=True, stop=True)
                for st in range(2):
                    e = pexp.tile([P, PAIR], F16, tag=f"ex{st}",
                                  name=f"ex{st}")
                    nc.scalar.activation(
                        e[:], scp[st][:],
                        mybir.ActivationFunctionType.Exp,
                        bias=ebias_col[:], scale=INV_SQRT_H)
                    cur_ex[st] = e

            def emit_epilogue(etch, av, rfa, rfb):
                # per stream: denominators + normalized oT
                for st in range(2):
                    rbct = pbig.tile([P, PAIR], F32, tag="big", name="rbct")
                    rbc = rbct[:, 0:FREE]
                    nc.tensor.matmul(rbc[0:1, :], ones16[:],
                                     rfa[st][:, 0:FREE],
                                     start=True, stop=False)
                    nc.tensor.matmul(rbc[0:1, :], ones16[:],
                                     rfa[st][:, FREE:PAIR],
                                     start=False, stop=False)
                    nc.tensor.matmul(rbc[0:1, :], ones16[:],
                                     rfb[st][:, 0:FREE],
                                     start=False, stop=False)
                    nc.tensor.matmul(rbc[0:1, :], ones16[:],
                                     rfb[st][:, FREE:PAIR],
                                     start=False, stop=True)
                    rinv_r = prv.tile([1, FREE], F32, tag="rinv_r",
                                      name="rinv_r")
                    nc.vector.reciprocal(rinv_r[:], rbc[0:1, :])
                    nc.tensor.matmul(rbc[:, :], ones_row[:], rinv_r[:],
                                     start=True, stop=True)
                    rbs = prv.tile([P, FREE], F32, tag="rbs", name="rbs")
                    nc.vector.tensor_copy(rbs[:], rbc[:, :])
                    nc.vector.tensor_mul(
                        oT_sb[:, st * T_NOISE + etch * FREE:
                              st * T_NOISE + (etch + 1) * FREE],
                        av[st][:], rbs[:])
                d_queue.extend((etch, ti, dh)
                               for ti in range(4) for dh in range(2))

            d_queue = []
            for tch in range(T_CHUNKS):
                av = [pav.tile([P, FREE], F32, tag=f"av{st}", name=f"av{st}")
                      for st in range(2)]
                # two denominator accumulators per stream: rfa on DVE,
                # rfb on the (otherwise idle) Pool engine
                rfa = [prf.tile([P, PAIR], F16, tag=f"rfa{st}",
                                name=f"rfa{st}") for st in range(2)]
                rfb = [prf.tile([P, PAIR], F16, tag=f"rfb{st}",
                                name=f"rfb{st}") for st in range(2)]
                qsl = [qT_sb[:, st * T_NOISE + tch * FREE:
                             st * T_NOISE + (tch + 1) * FREE]
                       for st in range(2)]
                prev_ex = [None, None]
                cur_ex = [None, None]
                na = [0, 0]
                nb = [0, 0]
                for p in range(SP_PAIRS + 1):
                    if p < SP_PAIRS:
                        emit_scores_exp(p, qsl, cur_ex)
                    if p >= 1:
                        q = p - 1
                        for u in range(2):
                            si = 2 * q + u
                            for st in range(2):
                                nc.tensor.matmul(
                                    av[st][:],
                                    v_sb[:, si * P:(si + 1) * P],
                                    prev_ex[st][:, u * FREE:(u + 1) * FREE],
                                    start=(q == 0 and u == 0),
                                    stop=(q == SP_PAIRS - 1 and u == 1))
                        for st in range(2):
                            e = prev_ex[st]
                            # Pool engine takes a third of the fold chain,
                            # but not the tail (its op latency would delay
                            # the epilogue's denominator matmuls)
                            if q % 3 == 2 and q <= 20:
                                if nb[st] == 0:
                                    nc.gpsimd.tensor_copy(rfb[st][:], e[:])
                                else:
                                    nc.gpsimd.tensor_add(rfb[st][:],
                                                         rfb[st][:], e[:])
                                nb[st] += 1
                            else:               # DVE accumulator
                                if na[st] == 0:
                                    nc.vector.tensor_copy(rfa[st][:], e[:])
                                else:
                                    nc.vector.tensor_add(rfa[st][:],
                                                         rfa[st][:], e[:])
                                na[st] += 1
                        # interleave previous chunk's o-projection
                        if d_queue and p >= 2 and p % 3 == 2:
                            emit_d_piece(*d_queue.pop(0))
                    prev_ex = list(cur_ex)
                emit_epilogue(tch, av, rfa, rfb)
            for piece in d_queue:
                emit_d_piece(*piece)


def _get_program(reps=1):
    key = f"prog{reps}"
    if key not in _CACHE:
        _CACHE[key] = _build_program(reps)
    return _CACHE[key]


def prepare_in_maps(x_noise, target_hidden, Wq, Wk, Wv, Wo, q_scale, k_scale,
                    noise_positions, ctx_positions):
    x_noise = np.asarray(x_noise, dtype=np.float32)
    target_hidden = np.asarray(target_hidden, dtype=np.float32)
    Wq = np.asarray(Wq, dtype=np.float32)
    Wk = np.asarray(Wk, dtype=np.float32)
    Wv = np.asarray(Wv, dtype=np.float32)
    Wo = np.asarray(Wo, dtype=np.float32)
    q_scale = np.asarray(q_scale, dtype=np.float32)
    k_scale = np.asarray(k_scale, dtype=np.float32)

    x_all = np.concatenate([target_hidden, x_noise], axis=0)       # (S, D)
    # xs[p, d, s] = x_all[s, d*128+p]
    xs = np.ascontiguousarray(
        x_all.T.reshape(D_TILES, P, S_ALL).transpose(1, 0, 2)
    ).astype(np.float16)
    pos_all = np.concatenate(
        [np.asarray(ctx_positions), np.asarray(noise_positions)]
    ).astype(np.float64)
    inv_freq = ROPE_THETA ** (-np.arange(HALF, dtype=np.float64) * 2.0 / H)
    ang = pos_all[:, None] * inv_freq[None, :]                     # (S, 64)
    # sint[p, si*64+j] = sin(pos[si*128+p] * invf[j])
    sint = np.ascontiguousarray(
        np.sin(ang).reshape(S_TILES, P, HALF).transpose(1, 0, 2)
        .reshape(P, S_TILES * HALF)).astype(np.float16)
    cost = np.ascontiguousarray(
        np.cos(ang).reshape(S_TILES, P, HALF).transpose(1, 0, 2)
        .reshape(P, S_TILES * HALF)).astype(np.float16)
    qscaleb = np.ascontiguousarray(np.broadcast_to(q_scale, (P, H)))
    kscaleb = np.ascontiguousarray(np.broadcast_to(k_scale, (P, H)))

    in_maps = []
    for c in range(N_CORES):
        wkvq = np.concatenate(
            [Wk[:, c, :], Wv[:, c, :],
             Wq[:, 2 * c, :], Wq[:, 2 * c + 1, :]], axis=1)        # (D, 512)
        wkvq = np.ascontiguousarray(
            wkvq.reshape(D_TILES, P, 4 * H).transpose(1, 0, 2)
        ).astype(np.float16)                                        # (P,16,512)
        wob = np.ascontiguousarray(
            Wo[2 * c:2 * c + 2].transpose(1, 0, 2)
        ).astype(np.float16)                                        # (P,2,D)
        in_maps.append({
            "xs": xs, "wkvq": wkvq, "wob": wob,
            "sint": sint, "cost": cost,
            "qscaleb": qscaleb, "kscaleb": kscaleb,
        })
    return in_maps


def kernel(**inputs):
    in_maps = prepare_in_maps(**inputs)
    nc, out_name = _get_program()
    res = run_bass_kernel_spmd(nc, in_maps, core_ids=list(range(N_CORES)))
    acc = np.zeros((T_NOISE, D), dtype=np.float32)
    for r in res.results:
        acc += r[out_name].astype(np.float32)
    return acc


def run_traced(inputs, **kw):
    """Run once with NTFF tracing; returns BassKernelResults (exec_time_ns)."""
    in_maps = prepare_in_maps(**inputs)
    nc, out_name = _get_program()
    return run_bass_kernel_spmd(nc, in_maps, core_ids=list(range(N_CORES)),
                                trace=True, **kw)


# revision 33
# speedup vs baseline: 1.0166x; 1.0044x over previous
"""DFlashAttention Trainium2 kernel (8-core tensor-parallel over attention heads).

Shapes (hardcoded): D=2048, N=16 q-heads, K=8 kv-heads, H=128,
T_NOISE=2048 (query tokens), T_CTX=4096, S=6144 (kv tokens).

Sharding: core c owns q-heads {2c, 2c+1} and kv-head c (GQA groups=2).
Each core computes a partial (T, D) output (its 2 heads' slice of the
o-projection contraction); the host sums the 8 partials (TP unshard).

v2 design (all matmul operands fp16):
  - Phase A (merged QKV proj): x streamed once as fp16 in [128,16,1024]
    chunks (one DMA per chunk).  ctx chunks compute k|v (256-wide moving),
    noise chunks compute k|v|q0|q1 in one 512-wide moving matmul.
    RMS-norm + RoPE in token-partition layout, PE transpose -> kT/qT [h,s].
  - Phase C (attention): two head-streams software-pipelined per t-chunk:
    PE issue order per step p: scores(p) for both streams, then AV(p-1),
    so the PE never sits behind ACT's exp.  exp = e^(score/sqrt(H) - 6.6)
    -> fp16 (bias keeps probs in fp16 range; it cancels in normalization).
    Softmax denominators: fp16 tree-fold of prob tiles on DVE (2x mode)
    + one ones-matmul on the folded tile (kills the per-s-tile rowsum
    matmul chain of v1).  Normalization fused into the av->oT copy via a
    rank-1 broadcast matmul of 1/r.
  - Phase D (o-proj): per t-chunk, both heads accumulate into one PSUM
    bank group; DMA straight PSUM->HBM.  No vector-engine work.
"""

import sys

for _p in ("/opt/trn_rl_repo", "/root/.axon_site/_ro/trn_rl_repo"):
    if _p not in sys.path:
        sys.path.append(_p)

import math
import numpy as np

import concourse.bass as bass
import concourse.tile as tile
from concourse import bacc
from concourse import mybir
from concourse.bass_utils import run_bass_kernel_spmd
from concourse.masks import make_identity

D = 2048
N_HEADS = 16
K_HEADS = 8
H = 128
T_NOISE = 2048
T_CTX = 4096
S_ALL = T_CTX + T_NOISE          # 6144
EPS = 1e-6
ROPE_THETA = 1e6
N_CORES = 8
HEADS_PER_CORE = N_HEADS // N_CORES   # 2

P = 128                       # partition dim
HALF = H // 2                 # 64
S_TILES = S_ALL // P          # 48
T_TILES = T_NOISE // P        # 16
NOISE_TILE0 = T_CTX // P      # 32  (noise tokens are s-tiles 32..47)
D_TILES = D // P              # 16
FREE = 512
PAIR = 2 * FREE               # 1024
CHUNK = 1024                  # tokens per x DMA chunk
N_CHUNKS = S_ALL // CHUNK     # 6 (chunks 4,5 are the noise tokens)
T_CHUNKS = T_NOISE // FREE    # 4
SP_PAIRS = S_TILES // 2       # 24 score pairs per (head, t-chunk)

F32 = mybir.dt.float32
F16 = mybir.dt.float16

TWO_PI = 2.0 * math.pi
INV_SQRT_H = 1.0 / math.sqrt(H)
EXP_BIAS = -6.6               # e^(13.69-6.6)*48*1.025 < 65504 (fp16 safe)

_CACHE = {}


def _build_program(reps=1):
    nc = bacc.Bacc("TRN2", target_bir_lowering=False, debug=False,
                   num_devices=N_CORES)

    # xs[p, d, s] = x_all[s, d*128+p]  (host pre-swizzled)
    xs = nc.dram_tensor("xs", [P, D_TILES, S_ALL], F16,
                        kind="ExternalInput").ap()
    # wkvq[p, d, :] = [Wk | Wv | Wq0 | Wq1][d*128+p, :]
    wkvq = nc.dram_tensor("wkvq", [P, D_TILES, 4 * H], F16,
                          kind="ExternalInput").ap()
    # wob[p, j, :] = Wo[head j][p, :]
    wob = nc.dram_tensor("wob", [P, HEADS_PER_CORE, D], F16,
                         kind="ExternalInput").ap()
    # host-precomputed RoPE tables: sint[p, si*64+j] = sin(pos[si*128+p]*invf[j])
    sint = nc.dram_tensor("sint", [P, S_TILES * HALF], F16,
                          kind="ExternalInput").ap()
    cost = nc.dram_tensor("cost", [P, S_TILES * HALF], F16,
                          kind="ExternalInput").ap()
    qscaleb = nc.dram_tensor("qscaleb", [P, H], F32,
                             kind="ExternalInput").ap()
    kscaleb = nc.dram_tensor("kscaleb", [P, H], F32,
                             kind="ExternalInput").ap()
    out = nc.dram_tensor("out", [T_NOISE, D], F16, kind="ExternalOutput").ap()

    with tile.TileContext(nc) as tc:
        for rep in range(reps):
            _emit(nc, tc, xs, wkvq, wob, sint, cost, qscaleb, kscaleb,
                  out, pfx=f"r{rep}_")
    nc.compile()
    return nc, "out"


def _emit(nc, tc, xs, wkvq, wob, sint, cost, qscaleb, kscaleb, out, pfx=""):
    import contextlib
    ctx = contextlib.ExitStack()
    with ctx:
        xp = ctx.enter_context(tc.tile_pool(name=pfx + "pa_x", bufs=2))
        const = ctx.enter_context(tc.tile_pool(name=pfx + "const", bufs=1))
        persist = ctx.enter_context(tc.tile_pool(name=pfx + "persist", bufs=1))

        # ---- chunk-0 x staging issued FIRST so phase A starts asap ----
        xst0 = [xp.tile([P, D_TILES // 2, CHUNK], F16, tag=f"xst{hf}",
                        name=f"xst{hf}") for hf in range(2)]
        for hf in range(2):
            nc.sync.dma_start(xst0[hf][:], xs[:, hf * 8:(hf + 1) * 8, 0:CHUNK])
        # weights in quarters so the first d-tiles' matmuls start early
        wkvq_sb = const.tile([P, D_TILES * 4 * H], F16, tag="wkvq")
        QW = D_TILES * 4 * H // 4
        for g in range(4):
            nc.sync.dma_start(wkvq_sb[:, g * QW:(g + 1) * QW],
                              wkvq[:, g * 4:(g + 1) * 4, :])

        # ---- constants ----
        ident = const.tile([P, P], F16, tag="ident")
        make_identity(nc, ident[:])
        ones16 = const.tile([P, 1], F16, tag="ones16")
        nc.vector.memset(ones16[:], 1.0)
        ones_row = const.tile([1, P], F32, tag="ones_row")
        nc.vector.memset(ones_row[:], 1.0)
        qsc_sb = const.tile([P, H], F32, tag="qsc")
        nc.sync.dma_start(qsc_sb[:], qscaleb[:])
        ksc_sb = const.tile([P, H], F32, tag="ksc")
        nc.sync.dma_start(ksc_sb[:], kscaleb[:])
        eps_col = const.tile([P, 1], F32, tag="eps")
        nc.vector.memset(eps_col[:], EPS)
        ebias_col = const.tile([P, 1], F32, tag="ebias")
        nc.vector.memset(ebias_col[:], EXP_BIAS)
        wo_sb = const.tile([P, HEADS_PER_CORE * D], F16, tag="wo")
        nc.sync.dma_start(wo_sb[:], wob[:])

        # ---- persistent activations ----
        sin_all = persist.tile([P, S_TILES * HALF], F16, tag="sin")
        cos_all = persist.tile([P, S_TILES * HALF], F16, tag="cos")
        nc.sync.dma_start(sin_all[:], sint[:])
        nc.sync.dma_start(cos_all[:], cost[:])
        kT_sb = persist.tile([P, S_ALL], F16, tag="kT")
        v_sb = persist.tile([P, S_ALL], F16, tag="v")     # [s-tile, h] blocks
        qT_sb = persist.tile([P, HEADS_PER_CORE * T_NOISE], F16, tag="qT")
        oT_sb = persist.tile([P, HEADS_PER_CORE * T_NOISE], F16, tag="oT")

        def norm_rope_transpose(src_psum, scale_sb, si, dst_sb, work, psum_t):
            """src_psum [P(tok),H] f32 -> rms-norm*scale -> rope -> transpose
            -> dst_sb [P(h), 128 tok] fp16. si = token-tile for positions."""
            sq = work.tile([P, H], F32, tag="sq")
            ssq = work.tile([P, 1], F32, tag="ssq")
            nc.scalar.activation(sq[:], src_psum,
                                 mybir.ActivationFunctionType.Square,
                                 accum_out=ssq[:])
            rms = work.tile([P, 1], F32, tag="rms")
            nc.scalar.activation(rms[:], ssq[:],
                                 mybir.ActivationFunctionType.Sqrt,
                                 bias=eps_col[:], scale=1.0 / H)
            rinv = work.tile([P, 1], F32, tag="rinv")
            nc.vector.reciprocal(rinv[:], rms[:])
            xn = work.tile([P, H], F16, tag="xn")
            nc.vector.scalar_tensor_tensor(
                xn[:], src_psum, rinv[:], scale_sb[:],
                mybir.AluOpType.mult, mybir.AluOpType.mult)
            co = cos_all[:, si * HALF:(si + 1) * HALF]
            sn = sin_all[:, si * HALF:(si + 1) * HALF]
            x1 = xn[:, 0:HALF]
            x2 = xn[:, HALF:H]
            t1 = work.tile([P, HALF], F16, tag="t1")
            t2 = work.tile([P, HALF], F16, tag="t2")
            xr = work.tile([P, H], F16, tag="xr")
            nc.vector.tensor_mul(t1[:], x1, co)
            nc.vector.tensor_mul(t2[:], x2, sn)
            nc.vector.tensor_sub(xr[:, 0:HALF], t1[:], t2[:])
            nc.vector.tensor_mul(t1[:], x2, co)
            nc.vector.tensor_mul(t2[:], x1, sn)
            nc.vector.tensor_add(xr[:, HALF:H], t1[:], t2[:])
            pt = psum_t.tile([P, P], F16, tag="pt")
            nc.tensor.transpose(pt[:], xr[:], ident[:])
            nc.scalar.copy(dst_sb, pt[:])

        # ---- Phase A: merged kvq projection -> kT, v, qT ----
        with tc.tile_pool(name=pfx + "pa_ps", bufs=3, space="PSUM") as pska, \
             tc.tile_pool(name=pfx + "pa_pt", bufs=2, space="PSUM") as pst, \
             tc.tile_pool(name=pfx + "pa_w", bufs=4) as work:
            def consume_pair(ps, si0, noi):
                for u in range(2):
                    si = si0 + u
                    nc.vector.tensor_copy(
                        v_sb[:, si * P:(si + 1) * P], ps[u][:, H:2 * H])
                    norm_rope_transpose(
                        ps[u][:, 0:H], ksc_sb, si,
                        kT_sb[:, si * P:(si + 1) * P], work, pst)
                    if noi:
                        ti = si - NOISE_TILE0
                        for hh in range(HEADS_PER_CORE):
                            norm_rope_transpose(
                                ps[u][:, (2 + hh) * H:(3 + hh) * H],
                                qsc_sb, si,
                                qT_sb[:, hh * T_NOISE + ti * P:
                                      hh * T_NOISE + (ti + 1) * P],
                                work, pst)

            for c in range(N_CHUNKS):
                noise = c >= 4
                W = 4 * H if noise else 2 * H
                # x chunk staged in two halves so the first matmuls can
                # start before the whole chunk has landed
                if c == 0:
                    xst = xst0
                else:
                    xst = [xp.tile([P, D_TILES // 2, CHUNK], F16,
                                   tag=f"xst{hf}", name=f"xst{hf}")
                           for hf in range(2)]
                    for hf in range(2):
                        nc.sync.dma_start(
                            xst[hf][:],
                            xs[:, hf * 8:(hf + 1) * 8,
                               c * CHUNK:(c + 1) * CHUNK])
                # 8 token-tiles per chunk, processed in pairs (2-deep PSUM).
                # Consumption lags the matmul bursts by one pair so the
                # norm/rope/transpose chain never blocks the mm stream.
                for pairi in range(4):
                    ps = [pska.tile([P, 4 * H], F32, tag=f"ps{u}",
                                    name=f"ps{u}") for u in range(2)]
                    for d in range(D_TILES):
                        for u in range(2):
                            tok = pairi * 2 + u
                            nc.tensor.matmul(
                                ps[u][:, 0:W],
                                xst[d // 8][:, d % 8, tok * P:(tok + 1) * P],
                                wkvq_sb[:, d * 4 * H:d * 4 * H + W],
                                start=(d == 0), stop=(d == D_TILES - 1))
                    consume_pair(ps, c * 8 + pairi * 2, noise)

        # ---- Phase C + D: attention (2 head-streams) + o-projection ----
        # PSUM: one rotating 3-deep [P,1024] pool (6 banks) shared by the
        # score tiles of both streams, D's po pairs, and the epilogue rbc;
        # + av0,av1 accumulators (1 bank each).
        with tc.tile_pool(name=pfx + "pc_big", bufs=3, space="PSUM") as pbig, \
             tc.tile_pool(name=pfx + "pc_av", bufs=1, space="PSUM") as pav, \
             tc.tile_pool(name=pfx + "pc_ex", bufs=8) as pexp, \
             tc.tile_pool(name=pfx + "pc_rf", bufs=2) as prf, \
             tc.tile_pool(name=pfx + "pc_rv", bufs=2) as prv, \
             tc.tile_pool(name=pfx + "pc_ob", bufs=4) as posb:

            def emit_d_piece(dtch, ti, dh):
                """o-projection for t-tile ti, D-half dh of t-chunk dtch."""
                t0 = dtch * FREE + ti * P
                po = pbig.tile([P, PAIR], F32, tag="big", name="po")
                for st in range(2):
                    osl = oT_sb[:, st * T_NOISE + t0:st * T_NOISE + t0 + P]
                    for u in range(2):
                        nc.tensor.matmul(
                            po[:, u * FREE:(u + 1) * FREE], osl,
                            wo_sb[:, st * D + dh * PAIR + u * FREE:
                                  st * D + dh * PAIR + (u + 1) * FREE],
                            start=(st == 0), stop=(st == 1))
                for u in range(2):
                    ob = posb.tile([P, FREE], F16, tag="ob", name="ob")
                    nc.vector.tensor_copy(ob[:], po[:, u * FREE:(u + 1) * FREE])
                    nc.sync.dma_start(
                        out[t0:t0 + P,
                            dh * PAIR + u * FREE:dh * PAIR + (u + 1) * FREE],
                        ob[:])

            def emit_scores_exp(p, qsl, cur_ex):
                # scores for pair p; stationary kT tile shared between the
                # two streams (u-outer).  Fresh score tiles from the
                # rotating pool: the PE never waits for exp's read of the
                # previous pair.
                scp = [pbig.tile([P, PAIR], F32, tag="big",
                                 name=f"sc{st}") for st in range(2)]
                for u in range(2):
                    si = 2 * p + u
                    for st in range(2):
                        nc.tensor.matmul(
                            scp[st][:, u * FREE:(u + 1) * FREE],
                            kT_sb[:, si * P:(si + 1) * P], qsl[st],
                            start=True, stop=True)
                for st in range(2):
                    e = pexp.tile([P, PAIR], F16, tag=f"ex{st}",
                                  name=f"ex{st}")
                    nc.scalar.activation(
                        e[:], scp[st][:],
                        mybir.ActivationFunctionType.Exp,
                        bias=ebias_col[:], scale=INV_SQRT_H)
                    cur_ex[st] = e

            def emit_epilogue(etch, av, rfa, rfb):
                # per stream: denominators + normalized oT
                for st in range(2):
                    rbct = pbig.tile([P, PAIR], F32, tag="big", name="rbct")
                    rbc = rbct[:, 0:FREE]
                    nc.tensor.matmul(rbc[0:1, :], ones16[:],
                                     rfa[st][:, 0:FREE],
                                     start=True, stop=False)
                    nc.tensor.matmul(rbc[0:1, :], ones16[:],
                                     rfa[st][:, FREE:PAIR],
                                     start=False, stop=False)
                    nc.tensor.matmul(rbc[0:1, :], ones16[:],
                                     rfb[st][:, 0:FREE],
                                     start=False, stop=False)
                    nc.tensor.matmul(rbc[0:1, :], ones16[:],
                                     rfb[st][:, FREE:PAIR],
                                     start=False, stop=True)
                    rinv_r = prv.tile([1, FREE], F32, tag="rinv_r",
                                      name="rinv_r")
                    nc.vector.reciprocal(rinv_r[:], rbc[0:1, :])
                    nc.tensor.matmul(rbc[:, :], ones_row[:], rinv_r[:],
                                     start=True, stop=True)
                    rbs = prv.tile([P, FREE], F32, tag="rbs", name="rbs")
                    nc.vector.tensor_copy(rbs[:], rbc[:, :])
                    nc.vector.tensor_mul(
                        oT_sb[:, st * T_NOISE + etch * FREE:
                              st * T_NOISE + (etch + 1) * FREE],
                        av[st][:], rbs[:])
                d_queue.extend((etch, ti, dh)
                               for ti in range(4) for dh in range(2))

            d_queue = []
            for tch in range(T_CHUNKS):
                av = [pav.tile([P, FREE], F32, tag=f"av{st}", name=f"av{st}")
                      for st in range(2)]
                # two denominator accumulators per stream: rfa on DVE,
                # rfb on the (otherwise idle) Pool engine
                rfa = [prf.tile([P, PAIR], F16, tag=f"rfa{st}",
                                name=f"rfa{st}") for st in range(2)]
                rfb = [prf.tile([P, PAIR], F16, tag=f"rfb{st}",
                                name=f"rfb{st}") for st in range(2)]
                qsl = [qT_sb[:, st * T_NOISE + tch * FREE:
                             st * T_NOISE + (tch + 1) * FREE]
                       for st in range(2)]
                prev_ex = [None, None]
                cur_ex = [None, None]
                na = [0, 0]
                nb = [0, 0]
                for p in range(SP_PAIRS + 1):
                    if p < SP_PAIRS:
                        emit_scores_exp(p, qsl, cur_ex)
                    if p >= 1:
                        q = p - 1
                        for u in range(2):
                            si = 2 * q + u
                            for st in range(2):
                                nc.tensor.matmul(
                                    av[st][:],
                                    v_sb[:, si * P:(si + 1) * P],
                                    prev_ex[st][:, u * FREE:(u + 1) * FREE],
                                    start=(q == 0 and u == 0),
                                    stop=(q == SP_PAIRS - 1 and u == 1))
                        for st in range(2):
                            e = prev_ex[st]
                            # Pool engine takes a third of the fold chain,
                            # but not the tail (its op latency would delay
                            # the epilogue's denominator matmuls)
                            if q % 3 == 2 and q <= 20:
                                if nb[st] == 0:
                                    nc.gpsimd.tensor_copy(rfb[st][:], e[:])
                                else:
                                    nc.gpsimd.tensor_add(rfb[st][:],
                                                         rfb[st][:], e[:])
                                nb[st] += 1
                            else:               # DVE accumulator
                                if na[st] == 0:
                                    nc.vector.tensor_copy(rfa[st][:], e[:])
                                else:
                                    nc.vector.tensor_add(rfa[st][:],
                                                         rfa[st][:], e[:])
                                na[st] += 1
                        # interleave previous chunk's o-projection
                        if d_queue and p >= 2 and p % 3 == 2:
                            emit_d_piece(*d_queue.pop(0))
                    prev_ex = list(cur_ex)
                emit_epilogue(tch, av, rfa, rfb)
            for piece in d_queue:
                emit_d_piece(*piece)


def _get_program(reps=1):
    key = f"prog{reps}"
    if key not in _CACHE:
        _CACHE[key] = _build_program(reps)
    return _CACHE[key]


def prepare_in_maps(x_noise, target_hidden, Wq, Wk, Wv, Wo, q_scale, k_scale,
                    noise_positions, ctx_positions):
    x_noise = np.asarray(x_noise, dtype=np.float32)
    target_hidden = np.asarray(target_hidden, dtype=np.float32)
    Wq = np.asarray(Wq, dtype=np.float32)
    Wk = np.asarray(Wk, dtype=np.float32)
    Wv = np.asarray(Wv, dtype=np.float32)
    Wo = np.asarray(Wo, dtype=np.float32)
    q_scale = np.asarray(q_scale, dtype=np.float32)
    k_scale = np.asarray(k_scale, dtype=np.float32)

    x_all = np.concatenate([target_hidden, x_noise], axis=0)       # (S, D)
    # xs[p, d, s] = x_all[s, d*128+p]
    xs = np.ascontiguousarray(
        x_all.T.reshape(D_TILES, P, S_ALL).transpose(1, 0, 2)
    ).astype(np.float16)
    pos_all = np.concatenate(
        [np.asarray(ctx_positions), np.asarray(noise_positions)]
    ).astype(np.float64)
    inv_freq = ROPE_THETA ** (-np.arange(HALF, dtype=np.float64) * 2.0 / H)
    ang = pos_all[:, None] * inv_freq[None, :]                     # (S, 64)
    # sint[p, si*64+j] = sin(pos[si*128+p] * invf[j])
    sint = np.ascontiguousarray(
        np.sin(ang).reshape(S_TILES, P, HALF).transpose(1, 0, 2)
        .reshape(P, S_TILES * HALF)).astype(np.float16)
    cost = np.ascontiguousarray(
        np.cos(ang).reshape(S_TILES, P, HALF).transpose(1, 0, 2)
        .reshape(P, S_TILES * HALF)).astype(np.float16)
    qscaleb = np.ascontiguousarray(np.broadcast_to(q_scale, (P, H)))
    kscaleb = np.ascontiguousarray(np.broadcast_to(k_scale, (P, H)))

    in_maps = []
    for c in range(N_CORES):
        wkvq = np.concatenate(
            [Wk[:, c, :], Wv[:, c, :],
             Wq[:, 2 * c, :], Wq[:, 2 * c + 1, :]], axis=1)        # (D, 512)
        wkvq = np.ascontiguousarray(
            wkvq.reshape(D_TILES, P, 4 * H).transpose(1, 0, 2)
        ).astype(np.float16)                                        # (P,16,512)
        wob = np.ascontiguousarray(
            Wo[2 * c:2 * c + 2].transpose(1, 0, 2)
        ).astype(np.float16)                                        # (P,2,D)
        in_maps.append({
            "xs": xs, "wkvq": wkvq, "wob": wob,
            "sint": sint, "cost": cost,
            "qscaleb": qscaleb, "kscaleb": kscaleb,
        })
    return in_maps


def kernel(**inputs):
    in_maps = prepare_in_maps(**inputs)
    nc, out_name = _get_program()
    res = run_bass_kernel_spmd(nc, in_maps, core_ids=list(range(N_CORES)))
    acc = np.zeros((T_NOISE, D), dtype=np.float32)
    for r in res.results:
        acc += r[out_name].astype(np.float32)
    return acc


def run_traced(inputs, **kw):
    """Run once with NTFF tracing; returns BassKernelResults (exec_time_ns)."""
    in_maps = prepare_in_maps(**inputs)
    nc, out_name = _get_program()
    return run_bass_kernel_spmd(nc, in_maps, core_ids=list(range(N_CORES)),
                                trace=True, **kw)
